# revision 1
# baseline (speedup 1.0000x reference)
"""Trainium2 Bass kernel for nn_KVCacheHybrid (quantized KV-cache scatter-update).

Reference semantics (per cache, k and v independently):
  1. 4-bit affine quantize along L (scales/zeros reduce over B,H,D per l)
  2. dequantize, scatter new rows at input_pos, re-quantize, dequantize.

Key observations that shape this kernel:
  * After the first quantize/dequant round-trip, codes 0 and 15 are attained in
    every l-slice, so the second-pass min/max for non-updated l are exactly the
    dequant grid endpoints: mn2 = z1 - 8*s1, mx2 = z1 + 7*s1.  No second data
    reduction is needed.
  * For non-updated l the second-pass codes equal the first-pass codes, so
    out = q1 * s2 + mn2.  Per element the device only computes
    q1 = round((x - mn1) * (1/s1)) and the affine above.
  * Rows at input_pos depend only on k_val/v_val (0.5 MB) — computed exactly on
    the host and spliced into the gathered output.

Sharding: L axis across 8 cores (512 l's each).  The per-l reduction is then
fully core-local — no collectives.

Device layout: partition dim = l (128 per chunk), free dim = (16 heads x 128 d)
=> [128, 2048] fp32 tiles, 1 MiB DMAs.
"""

import numpy as np
from contextlib import ExitStack

import concourse.bass as bass
import concourse.bacc as bacc
import concourse.tile as tile
from concourse import mybir
from concourse.bass_utils import run_bass_kernel_spmd

F32 = mybir.dt.float32
ALU = mybir.AluOpType
AXIS = mybir.AxisListType
ACTF = mybir.ActivationFunctionType

B, H, L, D = 2, 32, 4096, 128
N_CORES = 8
LC = L // N_CORES          # 512 l's per core
LCHUNK = 128               # l's per partition-tile
HG = 16                    # heads per tile (free dim = HG*D = 2048)
MAGIC = float(np.float32(2 ** 23))   # round-to-nearest-even constant
C15 = float(np.float32(1.0 / 15.0))

_BUILD_CACHE = {}


def _build(lc=LC):
    """Builds the per-core SPMD program; identical on all cores."""
    nc = bacc.Bacc("TRN2", target_bir_lowering=False, debug=False,
                   num_devices=N_CORES)
    k = nc.dram_tensor("k", [B, H, lc, D], F32, kind="ExternalInput").ap()
    v = nc.dram_tensor("v", [B, H, lc, D], F32, kind="ExternalInput").ap()
    out = nc.dram_tensor("out", [2, B, H, lc, D], F32, kind="ExternalOutput").ap()

    n_chunks = lc // LCHUNK
    n_hg = H // HG

    with tile.TileContext(nc) as tc, ExitStack() as ctx:
        xpool = ctx.enter_context(tc.tile_pool(name="x", bufs=12))
        tpool = ctx.enter_context(tc.tile_pool(name="t", bufs=4))
        opool = ctx.enter_context(tc.tile_pool(name="o", bufs=5))
        ppool = ctx.enter_context(tc.tile_pool(name="p", bufs=2))
        cpool = ctx.enter_context(tc.tile_pool(name="c", bufs=2))

        n_groups = 2 * n_chunks
        group_no = 0
        tile_no = 0
        for ci, src in enumerate((k, v)):
            for lchunk in range(n_chunks):
                l0 = lchunk * LCHUNK
                # ---- load + per-tile partial min/max --------------------
                pmin = ppool.tile([128, B * n_hg], F32, tag="pmin")
                pmax = ppool.tile([128, B * n_hg], F32, tag="pmax")
                tiles = []
                j = 0
                for b in range(B):
                    for hg in range(n_hg):
                        x2 = xpool.tile([128, HG * D], F32, tag="x")
                        x3 = x2[:].rearrange("l (h d) -> l h d", h=HG)
                        src_ap = src[b, hg * HG:(hg + 1) * HG,
                                     l0:l0 + LCHUNK, :].rearrange("h l d -> l h d")
                        nc.sync.dma_start(out=x3, in_=src_ap)
                        nc.vector.tensor_reduce(pmin[:, j:j + 1], x2[:],
                                                axis=AXIS.X, op=ALU.min)
                        nc.vector.tensor_reduce(pmax[:, j:j + 1], x2[:],
                                                axis=AXIS.X, op=ALU.max)
                        tiles.append((x2, b, hg))
                        j += 1

                # ---- per-l constants (all [128,1]) ----------------------
                mn1 = cpool.tile([128, 1], F32, tag="mn1")
                mx1 = cpool.tile([128, 1], F32, tag="mx1")
                nc.vector.tensor_reduce(mn1[:], pmin[:], axis=AXIS.X, op=ALU.min)
                nc.vector.tensor_reduce(mx1[:], pmax[:], axis=AXIS.X, op=ALU.max)
                dd = cpool.tile([128, 1], F32, tag="dd")
                nc.vector.tensor_tensor(dd[:], mx1[:], mn1[:], op=ALU.subtract)
                s1 = cpool.tile([128, 1], F32, tag="s1")
                # s1 = max(d,1e-6) * (1/15) -- HW tensor_scalar has no divide;
                # differs from the reference's d/15 by <=1 ulp (rare boundary flips)
                nc.vector.tensor_scalar(s1[:], dd[:], 1e-6, C15,
                                        op0=ALU.max, op1=ALU.mult)
                inv1 = cpool.tile([128, 1], F32, tag="inv1")
                nc.vector.reciprocal(inv1[:], s1[:])
                a8 = cpool.tile([128, 1], F32, tag="a8")
                nc.vector.tensor_scalar(a8[:], s1[:], 8.0, None, op0=ALU.mult)
                z1 = cpool.tile([128, 1], F32, tag="z1")
                nc.vector.tensor_tensor(z1[:], mn1[:], a8[:], op=ALU.add)
                mn2 = cpool.tile([128, 1], F32, tag="mn2")
                nc.vector.tensor_tensor(mn2[:], z1[:], a8[:], op=ALU.subtract)
                b7 = cpool.tile([128, 1], F32, tag="b7")
                nc.vector.tensor_scalar(b7[:], s1[:], 7.0, None, op0=ALU.mult)
                mx2 = cpool.tile([128, 1], F32, tag="mx2")
                nc.vector.tensor_tensor(mx2[:], z1[:], b7[:], op=ALU.add)
                d2 = cpool.tile([128, 1], F32, tag="d2")
                nc.vector.tensor_tensor(d2[:], mx2[:], mn2[:], op=ALU.subtract)
                s2 = cpool.tile([128, 1], F32, tag="s2")
                nc.vector.tensor_scalar(s2[:], d2[:], 1e-6, C15,
                                        op0=ALU.max, op1=ALU.mult)
                nb1 = cpool.tile([128, 1], F32, tag="nb1")
                # nb1 = -(mn1 * inv1): bias for the fused ACT affine
                nc.vector.tensor_scalar(nb1[:], mn1[:], inv1[:, 0:1], -1.0,
                                        op0=ALU.mult, op1=ALU.mult)

                # ---- elementwise + store -------------------------------
                # stage1 (fused affine) + stage3 (fused affine) on ACT,
                # stage2 (magic round-to-nearest-even) on DVE, in place.
                # GPSIMD's stock ts/tt ucode measured ~15x slower than DVE,
                # and its SBUF-port sharing stalls DVE — keep Pool idle.
                # The last two groups run their affines on DVE instead:
                # at the tail DVE is idle while ACT is the critical path.
                tail = group_no >= n_groups - 2
                for x2, b, hg in tiles:
                    t = tpool.tile([128, HG * D], F32, tag="t")
                    if tail:
                        nc.vector.tensor_scalar(t[:], x2[:], mn1[:, 0:1],
                                                inv1[:, 0:1],
                                                op0=ALU.subtract, op1=ALU.mult)
                    else:
                        nc.scalar.activation(t[:], x2[:], ACTF.Identity,
                                             bias=nb1[:, 0:1], scale=inv1[:, 0:1])
                    # stage2 (round, magic-constant): one DVE ts, in place.
                    # (Tried as two chained ACT Identity adds for early
                    # tiles — measured slower; ACT's per-op cost dominates.)
                    nc.vector.tensor_scalar(t[:], t[:], MAGIC, MAGIC,
                                            op0=ALU.add, op1=ALU.subtract)
                    o = opool.tile([128, HG * D], F32, tag="o")
                    if tail:
                        nc.vector.tensor_scalar(o[:], t[:], s2[:, 0:1],
                                                mn2[:, 0:1],
                                                op0=ALU.mult, op1=ALU.add)
                    else:
                        nc.scalar.activation(o[:], t[:], ACTF.Identity,
                                             bias=mn2[:, 0:1], scale=s2[:, 0:1])
                    tile_no += 1
                    dst_ap = out[ci, b, hg * HG:(hg + 1) * HG,
                                 l0:l0 + LCHUNK, :].rearrange("h l d -> l h d")
                    nc.scalar.dma_start(
                        out=dst_ap,
                        in_=o[:].rearrange("l (h d) -> l h d", h=HG))
                group_no += 1

    nc.compile()
    return nc


def _get_nc(lc=LC):
    if lc not in _BUILD_CACHE:
        _BUILD_CACHE[lc] = _build(lc)
    return _BUILD_CACHE[lc]


def _host_fix_rows(out, cache_idx, val, input_pos):
    """Exact (fp32, reference-op-order) outputs for the scattered rows."""
    f32 = np.float32
    val = np.asarray(val, dtype=np.float32)
    pos = [int(p) for p in np.asarray(input_pos)]
    # last write wins for duplicate positions
    posmap = {}
    for i, p in enumerate(pos):
        posmap[p] = i
    for p, i in posmap.items():
        row = val[:, :, i, :]                       # [B,H,D]
        mn = row.min()
        mx = row.max()
        s2 = f32(max(mx - mn, f32(1e-6)) / f32(15))
        z2 = f32(mn + f32(s2 * f32(8)))
        t = ((row - mn) / s2).astype(np.float32)
        q = np.clip(np.round(t), 0, 15).astype(np.float32)
        out[cache_idx, :, :, p, :] = ((q - f32(8)) * s2).astype(np.float32) + z2


def kernel(k_cache_f, v_cache_f, k_val, v_val, input_pos):
    k_cache_f = np.asarray(k_cache_f, dtype=np.float32)
    v_cache_f = np.asarray(v_cache_f, dtype=np.float32)
    nc = _get_nc()
    in_maps = []
    for c in range(N_CORES):
        sl = slice(c * LC, (c + 1) * LC)
        in_maps.append({
            "k": np.ascontiguousarray(k_cache_f[:, :, sl, :]),
            "v": np.ascontiguousarray(v_cache_f[:, :, sl, :]),
        })
    res = run_bass_kernel_spmd(nc, in_maps, list(range(N_CORES)))
    out = np.concatenate([res.results[c]["out"] for c in range(N_CORES)], axis=3)
    _host_fix_rows(out, 0, k_val, input_pos)
    _host_fix_rows(out, 1, v_val, input_pos)
    return out



# revision 2
# speedup vs baseline: 1.0214x; 1.0214x over previous
"""Trainium2 Bass kernel for nn_KVCacheHybrid (quantized KV-cache scatter-update).

Reference semantics (per cache, k and v independently):
  1. 4-bit affine quantize along L (scales/zeros reduce over B,H,D per l)
  2. dequantize, scatter new rows at input_pos, re-quantize, dequantize.

Math shortcuts (proven by the earlier 243us baseline, rel err ~3e-4):
  * Second-pass min/max for non-updated l are the dequant grid endpoints:
    mn2 = z1 - 8*s1, mx2 = z1 + 7*s1 -> no second data reduction.
  * For non-updated l: out = q1 * s2 + mn2 with q1 = round((x - mn1)/s1).
  * Rows at input_pos depend only on k_val/v_val -> computed on host, spliced.

This version (vs that baseline):
  * l-major DRAM layout: host pre-transposes caches to [L, B*H*D] so every
    DMA line is 16KB contiguous (was 512B) -> full HBM rate, ~8x fewer
    descriptors, cheap triggers.
  * round folded into the ACT affine via the fp32->int8 output convert,
    which (measured on HW) is round-to-nearest-even with saturation --
    exactly jnp.round + clip.  Both elementwise passes run on ACT
    (fp32 -> int8 codes -> fp32), so DVE runs ONLY the min/max reductions
    and the per-l constant chain.
  * [128, 4096] half-row tiles, 6-deep input pool: the load->reduce->
    consts->act1 latency chain is ~3 tiles long, so 6 buffers keep the
    input DMA queue saturated (3 buffers measurably starved it).
  * act1 is gated on only inv1/nb1 (4 small ops after the reduces); the
    rest of the const chain is emitted later, feeding act3.

Sharding: L axis across 8 cores (512 l's each, per-l reduction core-local,
no collectives).
"""

import numpy as np
from contextlib import ExitStack

import concourse.bass as bass
import concourse.bacc as bacc
import concourse.tile as tile
from concourse import mybir
from concourse.bass_utils import run_bass_kernel_spmd

F32 = mybir.dt.float32
I8 = mybir.dt.int8
ALU = mybir.AluOpType
AXIS = mybir.AxisListType
ACTF = mybir.ActivationFunctionType

B, H, L, D = 2, 32, 4096, 128
FD = B * H * D             # 8192 elements per l-row
N_CORES = 8
LC = L // N_CORES          # 512 l-rows per core
PCHUNK = 128               # l-rows per tile (partition dim)
TCOLS = 4096               # columns per tile; 2 col-tiles per l-row
C15 = float(np.float32(1.0 / 15.0))

_BUILD_CACHE = {}


def _build(lc=LC):
    nc = bacc.Bacc("TRN2", target_bir_lowering=False, debug=False,
                   num_devices=N_CORES)
    k = nc.dram_tensor("k", [lc, FD], F32, kind="ExternalInput").ap()
    v = nc.dram_tensor("v", [lc, FD], F32, kind="ExternalInput").ap()
    ok = nc.dram_tensor("ok", [lc, FD], F32, kind="ExternalOutput").ap()
    ov = nc.dram_tensor("ov", [lc, FD], F32, kind="ExternalOutput").ap()
    n_chunks = lc // PCHUNK

    with tile.TileContext(nc) as tc, ExitStack() as ctx:
        xpool = ctx.enter_context(tc.tile_pool(name="x", bufs=6))
        qpool = ctx.enter_context(tc.tile_pool(name="q", bufs=4))
        opool = ctx.enter_context(tc.tile_pool(name="o", bufs=4))
        cpool = ctx.enter_context(tc.tile_pool(name="c", bufs=2))

        for src, dst in ((k, ok), (v, ov)):
            for ch in range(n_chunks):
                l0 = ch * PCHUNK
                rows = src[l0:l0 + PCHUNK, :]
                XA = xpool.tile([PCHUNK, TCOLS], F32, tag="x")
                XB = xpool.tile([PCHUNK, TCOLS], F32, tag="x")
                nc.sync.dma_start(out=XA[:], in_=rows[:, 0:TCOLS])
                nc.sync.dma_start(out=XB[:], in_=rows[:, TCOLS:FD])

                mna = cpool.tile([PCHUNK, 1], F32, tag="mna")
                mnb = cpool.tile([PCHUNK, 1], F32, tag="mnb")
                mxa = cpool.tile([PCHUNK, 1], F32, tag="mxa")
                mxb = cpool.tile([PCHUNK, 1], F32, tag="mxb")
                nc.vector.tensor_reduce(mna[:], XA[:], axis=AXIS.X, op=ALU.min)
                nc.vector.tensor_reduce(mxa[:], XA[:], axis=AXIS.X, op=ALU.max)
                nc.vector.tensor_reduce(mnb[:], XB[:], axis=AXIS.X, op=ALU.min)
                nc.vector.tensor_reduce(mxb[:], XB[:], axis=AXIS.X, op=ALU.max)
                mn1 = cpool.tile([PCHUNK, 1], F32, tag="mn1")
                mx1 = cpool.tile([PCHUNK, 1], F32, tag="mx1")
                nc.vector.tensor_tensor(mn1[:], mna[:], mnb[:], op=ALU.min)
                nc.vector.tensor_tensor(mx1[:], mxa[:], mxb[:], op=ALU.max)

                # minimal chain for act1: dd -> s1 -> inv1 -> nb1
                dd = cpool.tile([PCHUNK, 1], F32, tag="dd")
                nc.vector.tensor_tensor(dd[:], mx1[:], mn1[:], op=ALU.subtract)
                s1 = cpool.tile([PCHUNK, 1], F32, tag="s1")
                nc.vector.tensor_scalar(s1[:], dd[:], 1e-6, C15,
                                        op0=ALU.max, op1=ALU.mult)
                inv1 = cpool.tile([PCHUNK, 1], F32, tag="inv1")
                nc.vector.reciprocal(inv1[:], s1[:])
                nb1 = cpool.tile([PCHUNK, 1], F32, tag="nb1")
                # nb1 = -(mn1 * inv1): bias of the quantize affine
                nc.vector.tensor_scalar(nb1[:], mn1[:], inv1[:, 0:1], -1.0,
                                        op0=ALU.mult, op1=ALU.mult)

                # q = RNE(x*inv1 + nb1) via the int8 output convert
                QA = qpool.tile([PCHUNK, TCOLS], I8, tag="q")
                QB = qpool.tile([PCHUNK, TCOLS], I8, tag="q")
                nc.scalar.activation(QA[:], XA[:], ACTF.Identity,
                                     bias=nb1[:, 0:1], scale=inv1[:, 0:1])
                nc.scalar.activation(QB[:], XB[:], ACTF.Identity,
                                     bias=nb1[:, 0:1], scale=inv1[:, 0:1])

                # rest of the const chain, feeding act3 only
                a8 = cpool.tile([PCHUNK, 1], F32, tag="a8")
                nc.vector.tensor_scalar(a8[:], s1[:], 8.0, None, op0=ALU.mult)
                z1 = cpool.tile([PCHUNK, 1], F32, tag="z1")
                nc.vector.tensor_tensor(z1[:], mn1[:], a8[:], op=ALU.add)
                mn2 = cpool.tile([PCHUNK, 1], F32, tag="mn2")
                nc.vector.tensor_tensor(mn2[:], z1[:], a8[:], op=ALU.subtract)
                b7 = cpool.tile([PCHUNK, 1], F32, tag="b7")
                nc.vector.tensor_scalar(b7[:], s1[:], 7.0, None, op0=ALU.mult)
                mx2 = cpool.tile([PCHUNK, 1], F32, tag="mx2")
                nc.vector.tensor_tensor(mx2[:], z1[:], b7[:], op=ALU.add)
                d2 = cpool.tile([PCHUNK, 1], F32, tag="d2")
                nc.vector.tensor_tensor(d2[:], mx2[:], mn2[:], op=ALU.subtract)
                s2 = cpool.tile([PCHUNK, 1], F32, tag="s2")
                nc.vector.tensor_scalar(s2[:], d2[:], 1e-6, C15,
                                        op0=ALU.max, op1=ALU.mult)

                # out = q*s2 + mn2 (int8 codes read back as fp32)
                OA = opool.tile([PCHUNK, TCOLS], F32, tag="o")
                OB = opool.tile([PCHUNK, TCOLS], F32, tag="o")
                nc.scalar.activation(OA[:], QA[:], ACTF.Identity,
                                     bias=mn2[:, 0:1], scale=s2[:, 0:1])
                nc.scalar.dma_start(out=dst[l0:l0 + PCHUNK, 0:TCOLS], in_=OA[:])
                nc.scalar.activation(OB[:], QB[:], ACTF.Identity,
                                     bias=mn2[:, 0:1], scale=s2[:, 0:1])
                nc.scalar.dma_start(out=dst[l0:l0 + PCHUNK, TCOLS:FD], in_=OB[:])

    nc.compile()
    return nc


def _get_nc(lc=LC):
    if lc not in _BUILD_CACHE:
        _BUILD_CACHE[lc] = _build(lc)
    return _BUILD_CACHE[lc]


def make_in_maps(k_cache_f, v_cache_f):
    """l-major reshard: [B,H,L,D] -> per-core [LC, B*H*D] row blocks."""
    kt = np.ascontiguousarray(np.moveaxis(np.asarray(k_cache_f, np.float32), 2, 0)
                              ).reshape(L, FD)
    vt = np.ascontiguousarray(np.moveaxis(np.asarray(v_cache_f, np.float32), 2, 0)
                              ).reshape(L, FD)
    return [{"k": kt[c * LC:(c + 1) * LC], "v": vt[c * LC:(c + 1) * LC]}
            for c in range(N_CORES)]


def _host_fix_rows(out, cache_idx, val, input_pos):
    """Exact (fp32, reference-op-order) outputs for the scattered rows."""
    f32 = np.float32
    val = np.asarray(val, dtype=np.float32)
    pos = [int(p) for p in np.asarray(input_pos)]
    # last write wins for duplicate positions
    posmap = {}
    for i, p in enumerate(pos):
        posmap[p] = i
    for p, i in posmap.items():
        row = val[:, :, i, :]                       # [B,H,D]
        mn = row.min()
        mx = row.max()
        s2 = f32(max(mx - mn, f32(1e-6)) / f32(15))
        z2 = f32(mn + f32(s2 * f32(8)))
        t = ((row - mn) / s2).astype(np.float32)
        q = np.clip(np.round(t), 0, 15).astype(np.float32)
        out[cache_idx, :, :, p, :] = ((q - f32(8)) * s2).astype(np.float32) + z2


def kernel(k_cache_f, v_cache_f, k_val, v_val, input_pos):
    nc = _get_nc()
    in_maps = make_in_maps(k_cache_f, v_cache_f)
    res = run_bass_kernel_spmd(nc, in_maps, list(range(N_CORES)))
    out = np.empty((2, B, H, L, D), dtype=np.float32)
    for c in range(N_CORES):
        sl = slice(c * LC, (c + 1) * LC)
        out[0, :, :, sl, :] = res.results[c]["ok"].reshape(
            LC, B, H, D).transpose(1, 2, 0, 3)
        out[1, :, :, sl, :] = res.results[c]["ov"].reshape(
            LC, B, H, D).transpose(1, 2, 0, 3)
    _host_fix_rows(out, 0, k_val, input_pos)
    _host_fix_rows(out, 1, v_val, input_pos)
    return out
